# revision 3
# baseline (speedup 1.0000x reference)
"""Trainium2 Bass kernel for nn_ALLonBert_v3 (segment_reduce + tiny classifier).

Computation (per batch row b):
  means[k, :] = mean of sequence_outputs[b, t, :] over tokens t in segment k
  logits[b, k, c] = means[k, :] @ W[c, :] + b[c]

Device strategy (pure data-parallel, 8 batch rows per core, no collectives):
  - Host builds a one-hot assignment matrix A[t, j] (token t belongs to
    segment k), mirroring the host-side SEP scan the original module performs.
    The 64 lhsT columns cover 2 classes x (2 rows of a pair) x 16 segments:
    the class dim is a duplicated one-hot so each PSUM tile carries the sums
    twice (partition p = 32*class + 16*half + k), letting the classifier do a
    single fused scalar_tensor_tensor per PSUM tile (per-partition W table)
    instead of one pass per class.  Matmul cost only scales with rhs free
    size, so the duplication is free on the PE.
  - The x stream is compressed host-side: hidden columns are sorted by
    classifier energy |W[:,h]|^2; the top 256 go in bf16 (xhi), the bottom
    512 in per-column-scaled fp8 e3m4 (xlo), scales folded into the W table.
    HBM traffic drops 12.58 -> 4.19 MB/core; quantization rel-err ~8e-3
    (tolerance 2e-2).
  - Segment sums accumulate in f32 PSUM per row-pair:
      m1[64p, 512] += A2[t, :].T @ xlo[t, :]   (fp8e3 x fp8e3)
      m2[64p, 256] += A2[t, :].T @ xhi[t, :]   (fp8e4 x bf16)
  - DVE classifier reads sums straight from PSUM per pair: one stt per tile
    computes (sums * invcnt) * W' and its free-axis sum; one add joins the
    lo/hi partials into logits [64, 1].
Steady state is jointly PE/DMA-bound (~8.7 us DMA, ~10.3 us PE per core).
"""

import sys

for _p in ("/opt/trn_rl_repo", "/opt/pypackages"):
    if _p not in sys.path:
        sys.path.insert(0, _p)

import ml_dtypes
import numpy as np

import concourse.bacc as bacc
import concourse.mybir as mybir
import concourse.tile as tile
from concourse.bass_utils import run_bass_kernel_spmd

B, S, H, NSEG = 64, 512, 768, 16
NCORES = 8
RPC = B // NCORES       # batch rows per core = 8
P = 128                 # partitions
NCH = S // P            # token chunks per row = 4
NLO = 512               # low-|W| columns, fp8 e3m4
NHI = H - NLO           # high-|W| columns, bf16
NPAIR = RPC // 2

F32 = mybir.dt.float32
BF16 = mybir.dt.bfloat16
FP8E4 = mybir.dt.float8e4
FP8E3 = mybir.dt.float8e3
NPBF16 = ml_dtypes.bfloat16
NPFP8E4 = ml_dtypes.float8_e4m3
NPFP8E3 = ml_dtypes.float8_e3m4

_graph_cache = {}


def _build_graph(reps=1, lo_dt=FP8E3, xbufs=6):
    nc = bacc.Bacc("TRN2", target_bir_lowering=False, debug=False,
                   num_devices=NCORES)

    np_lo = {FP8E3: "e3", BF16: "bf"}[lo_dt]
    lo_bytes = 1 if lo_dt == FP8E3 else 2
    xlo_ext = nc.declare_dram_parameter("xlo", [RPC * S, NLO], lo_dt,
                                        isOutput=False)
    xhi_ext = nc.declare_dram_parameter("xhi", [RPC * S, NHI], BF16,
                                        isOutput=False)
    alo_ext = nc.declare_dram_parameter("alo", [P, RPC * NCH * 64], lo_dt,
                                        isOutput=False)
    ahi_ext = nc.declare_dram_parameter("ahi", [P, RPC * NCH * 64], FP8E4,
                                        isOutput=False)
    w_ext = nc.declare_dram_parameter("wtab", [64, H], F32, isOutput=False)
    ic_ext = nc.declare_dram_parameter("invcnt", [64, NPAIR], F32,
                                       isOutput=False)
    out_ext = nc.declare_dram_parameter("out", [64, NPAIR], F32, isOutput=True)

    # x[(r c p), h] -> [r][p][c h]
    xlov = xlo_ext.ap().rearrange("(r c p) h -> r p c h", r=RPC, c=NCH, p=P)
    xhiv = xhi_ext.ap().rearrange("(r c p) h -> r p c h", r=RPC, c=NCH, p=P)

    with tile.TileContext(nc) as tc:
        with (
            tc.tile_pool(name="consts", bufs=1) as consts,
            tc.tile_pool(name="xp", bufs=xbufs) as xp,
            tc.tile_pool(name="ps1", bufs=3, space="PSUM") as ps1,
            tc.tile_pool(name="ps2", bufs=3, space="PSUM") as ps2,
            tc.tile_pool(name="tmp", bufs=2) as tmpp,
        ):
            alo_sb = consts.tile([P, RPC * NCH * 64], lo_dt)
            nc.sync.dma_start(out=alo_sb[:], in_=alo_ext.ap())
            ahi_sb = consts.tile([P, RPC * NCH * 64], FP8E4)
            nc.sync.dma_start(out=ahi_sb[:], in_=ahi_ext.ap())
            ic_sb = consts.tile([64, NPAIR], F32)
            w_sb = consts.tile([64, H], F32)
            cls_done = []

            def emit_cls_consts():
                if cls_done:
                    return
                cls_done.append(1)
                nc.sync.dma_start(out=ic_sb[:], in_=ic_ext.ap())
                nc.sync.dma_start(out=w_sb[:], in_=w_ext.ap())

            for rep in range(reps):
              row_lo = {}
              row_hi = {}
              logits_sb = tmpp.tile([64, NPAIR], F32, tag="logits")
              # Chunk-split the last pair's rows: the final pair's matmuls
              # then trail the stream by one chunk instead of waiting on
              # whole rows, shortening the serial tail after the last byte.
              for r in range(RPC):
                split = r >= RPC - 2
                if split:
                    los, his = [], []
                    for c in range(NCH):
                        xc = xp.tile([P, NLO], lo_dt, tag="xloc", bufs=8)
                        nc.sync.dma_start(out=xc[:], in_=xlov[r][:, c, :])
                        los.append((xc, 0))
                    for c in range(NCH):
                        xc = xp.tile([P, NHI], BF16, tag="xhic", bufs=8)
                        nc.sync.dma_start(out=xc[:], in_=xhiv[r][:, c, :])
                        his.append((xc, 0))
                    row_lo[r] = los
                    row_hi[r] = his
                else:
                    xlt = xp.tile([P, NCH * NLO], lo_dt, tag="xlo")
                    nc.sync.dma_start(
                        out=xlt[:].rearrange("p (c h) -> p c h", c=NCH),
                        in_=xlov[r])
                    xht = xp.tile([P, NCH * NHI], BF16, tag="xhi")
                    nc.sync.dma_start(
                        out=xht[:].rearrange("p (c h) -> p c h", c=NCH),
                        in_=xhiv[r])
                    row_lo[r] = [(xlt, c * NLO) for c in range(NCH)]
                    row_hi[r] = [(xht, c * NHI) for c in range(NCH)]
                if r == 1 and rep == 0:
                    emit_cls_consts()
                while True:
                    pairs = [g for g in range(NPAIR)
                             if 2 * g in row_lo and 2 * g + 1 in row_lo]
                    if not pairs:
                        break
                    g = pairs[0]
                    m1 = ps1.tile([64, NLO], F32)
                    m2 = ps2.tile([64, NHI], F32)
                    for half in range(2):
                        r = 2 * g + half
                        los = row_lo.pop(r)
                        his = row_hi.pop(r)
                        first = half == 0
                        last = half == 1
                        for c in range(NCH):
                            lhslo = alo_sb[:, (r * NCH + c) * 64:
                                           (r * NCH + c + 1) * 64]
                            lhshi = ahi_sb[:, (r * NCH + c) * 64:
                                           (r * NCH + c + 1) * 64]
                            (t1, o1) = los[c]
                            (t2, o2) = his[c]
                            nc.tensor.matmul(m1[:], lhslo,
                                             t1[:, o1:o1 + NLO],
                                             start=first and c == 0,
                                             stop=last and c == NCH - 1)
                            nc.tensor.matmul(m2[:], lhshi,
                                             t2[:, o2:o2 + NHI],
                                             start=first and c == 0,
                                             stop=last and c == NCH - 1)
                    # Classifier for this pair, straight from PSUM: one fused
                    # stt per tile computes (sums * invcnt) * W' and its
                    # free-axis sum (the class is on the partition dim, so a
                    # single pass covers both classes).
                    icg = ic_sb[0:64, g:g + 1]
                    rs = []
                    for mt, width, off, key in ((m1, NLO, 0, "r1"),
                                                (m2, NHI, NLO, "r2")):
                        pr = tmpp.tile([64, width], F32, tag=f"pr_{key}")
                        racc = tmpp.tile([64, 1], F32, tag=f"acc_{key}",
                                         bufs=4)
                        nc.vector.scalar_tensor_tensor(
                            out=pr[:], in0=mt[:], scalar=icg,
                            in1=w_sb[0:64, off:off + width],
                            op0=mybir.AluOpType.mult,
                            op1=mybir.AluOpType.mult,
                            accum_out=racc[:])
                        rs.append(racc)
                    nc.vector.tensor_add(out=logits_sb[0:64, g:g + 1],
                                         in0=rs[0][:], in1=rs[1][:])
              nc.sync.dma_start(out=out_ext.ap(), in_=logits_sb[:])

    nc.compile()
    return nc


def _get_graph(reps=1, lo_dt=FP8E3, xbufs=6):
    key = (reps, lo_dt, xbufs)
    if key not in _graph_cache:
        _graph_cache[key] = _build_graph(reps, lo_dt, xbufs)
    return _graph_cache[key]


def _segment_onehot(sep_positions: np.ndarray):
    """One-hot A[b, t, k] (reference semantics) and counts [b, k]."""
    t = np.arange(S)
    sep = np.asarray(sep_positions)
    seg_id = (t[None, None, :] >= sep[:, :, None]).sum(axis=1)        # [B, S]
    is_sep = (t[None, None, :] == sep[:, :, None]).any(axis=1)        # [B, S]
    valid = (t[None, :] >= 1) & (~is_sep) & (seg_id < NSEG)
    seg_clipped = np.where(valid, seg_id, NSEG)
    a = (seg_clipped[:, :, None] == np.arange(NSEG)[None, None, :])
    a = a.astype(np.float32)                                          # [B, S, NSEG]
    cnts = a.sum(axis=1)                                              # [B, NSEG]
    return a, cnts


def _pack_a(a_onehot_rows):
    """[RPC, S, NSEG] one-hot -> [P, RPC*NCH*64] lhsT blocks.

    Column j of block (r, c): class = j//32, half = (j%32)//16, k = j%16;
    holds onehot[r, t, k] when half == r%2 else 0 (the partner row's
    columns), duplicated across the two class blocks.
    """
    ach = a_onehot_rows.reshape(RPC, NCH, P, NSEG)        # [r, c, p, k]
    apad = np.zeros((RPC, NCH, P, 64), dtype=np.float32)
    for r in range(RPC):
        off = (r % 2) * NSEG
        apad[r, :, :, off:off + NSEG] = ach[r]
        apad[r, :, :, 32 + off:32 + off + NSEG] = ach[r]
    return apad.transpose(2, 0, 1, 3).reshape(P, RPC * NCH * 64)


def make_in_maps(sequence_outputs, sep_positions, W, lo_dt="e3"):
    x = np.ascontiguousarray(sequence_outputs, dtype=np.float32)
    w = np.ascontiguousarray(W, dtype=np.float32)
    a_onehot, cnts = _segment_onehot(sep_positions)
    inv = (1.0 / np.maximum(cnts, 1.0)).astype(np.float32)            # [B, NSEG]

    # Order hidden columns by classifier energy: top NHI stay bf16, the
    # remaining NLO are fp8 e3m4 with per-column scales folded into W'.
    perm = np.argsort(-(w ** 2).sum(axis=0), kind="stable")
    hi_cols = perm[:NHI]
    lo_cols = perm[NHI:]
    xhi = x[:, :, hi_cols].astype(NPBF16)                             # [B,S,NHI]
    xl = x[:, :, lo_cols]
    if lo_dt == "e3":
        absmax = np.abs(xl).reshape(-1, NLO).max(axis=0)
        s = np.where(absmax > 0, 15.0 / np.maximum(absmax, 1e-30), 1.0)
        s = s.astype(np.float32)
        xlo = (xl * s).astype(NPFP8E3)                                # [B,S,NLO]
        w_lo = w[:, lo_cols] / s
    else:
        xlo = xl.astype(NPBF16)
        w_lo = w[:, lo_cols]
    # W table [64, H]: partition p = 32*class + k2; every k2 row repeats the
    # class's W row, columns in [lo | hi] order.
    wmix = np.concatenate([w_lo, w[:, hi_cols]], axis=1)              # [2, H]
    wtab = np.repeat(wmix, 32, axis=0).astype(np.float32)             # [64, H]

    in_maps = []
    for m in range(NCORES):
        rows = slice(m * RPC, (m + 1) * RPC)
        apack = _pack_a(a_onehot[rows])
        # invcnt [64, NPAIR]: partition p = 32*class + 16*half + k, col = pair
        icv = inv[rows].reshape(NPAIR, 2, NSEG)          # [pair, half, k]
        icv = icv.reshape(NPAIR, 32).T                   # [32, NPAIR]
        icv = np.concatenate([icv, icv], axis=0)         # [64, NPAIR]
        in_maps.append({
            "xlo": np.ascontiguousarray(xlo[rows].reshape(RPC * S, NLO)),
            "xhi": np.ascontiguousarray(xhi[rows].reshape(RPC * S, NHI)),
            "alo": np.ascontiguousarray(
                apack.astype(NPFP8E3 if lo_dt == "e3" else NPBF16)),
            "ahi": np.ascontiguousarray(apack.astype(NPFP8E4)),
            "wtab": wtab,
            "invcnt": np.ascontiguousarray(icv.astype(np.float32)),
        })
    return in_maps


def kernel(sequence_outputs, sep_positions, W, b):
    bias = np.asarray(b, dtype=np.float32)
    in_maps = make_in_maps(sequence_outputs, sep_positions, W)
    nc = _get_graph()
    res = run_bass_kernel_spmd(nc, in_maps, core_ids=list(range(NCORES)))
    out = np.zeros((B, NSEG, 2), dtype=np.float32)
    for m in range(NCORES):
        o = res.results[m]["out"]                        # [64, NPAIR]
        o = o.reshape(2, 2, NSEG, NPAIR)                 # [class, half, k, pair]
        for g in range(NPAIR):
            for half in range(2):
                out[m * RPC + 2 * g + half] = o[:, half, :, g].T
    return out + bias[None, None, :]
